# revision 18
# baseline (speedup 1.0000x reference)
"""Causal self-attention (B=4, S=2048, D=1024, H=16) on 8 TRN2 NeuronCores.

Sharding (hybrid batch x heads): core c handles batch b = c//2 and head half
h = c%2 (8 heads, 512 channels).

Per-core pipeline (all matmul data fp16, accumulation fp32 in PSUM):
  qkvT slice = (w_qkv_local^T x^T) via PE, weight-stationary with rg-paired
  reuse;  V in natural [row, (h, hd)] layout.
  Attention per head-pair (even head on partitions 0-63, odd on 64-127):
    QK^T: two 64-contraction matmuls issued as concurrent PE row-tiles
      (tile_position (0,0) / (64,0)) into one two-bank wide PSUM tile.
    exp on ScalarE over the wide [128, 1024] tile (scale=1/8 fused).
    causal mask multiply on GpSimd (diagonal tiles only).
    running softmax denominator: DVE accumulates exp tiles; GpSimd
      partition_all_reduce gives Z broadcast to all partitions; 1/Z =
      exp(-ln Z) on ScalarE (ln+exp share one ACT table set - no reloads).
    AV: two M=64 matmuls as concurrent PE column-tiles ((0,0)/(0,64)),
      accumulating over k tiles; final y^T = av * (1/Z) on DVE.
  proj: y^T-stationary matmuls, rg-paired; b_proj and the V-bias
  contribution are folded in on the host (softmax rows sum to 1).
"""

import numpy as np
import ml_dtypes
from contextlib import ExitStack

import concourse.bass as bass
import concourse.bass_isa as bass_isa
import concourse.mybir as mybir
import concourse.tile as tile
from concourse import bacc
from concourse.bass_utils import run_bass_kernel_spmd

dt = mybir.dt
AF = mybir.ActivationFunctionType

B, S, D, H, HD = 4, 2048, 1024, 16, 64
NCORES = 8
HPC = 8            # heads per core
DL = HPC * HD      # local channel width (512)
P = 128
NQG = S // 512     # q groups of 512          -> 4
NDC = D // P       # D chunks of 128          -> 8
NRT = S // P       # row tiles for V / proj   -> 16

MM_DT = dt.float16
NP_MM = ml_dtypes.float16 if hasattr(ml_dtypes, "float16") else np.float16

_CACHE = {}


def _emit(nc, tc, ctx, io):
    xt, wqkv, wproj, bqk, maskd, out_p = io

    const = ctx.enter_context(tc.tile_pool(name="const", bufs=1))
    ptp = ctx.enter_context(tc.tile_pool(name="ptp", bufs=8))
    rcpp = ctx.enter_context(tc.tile_pool(name="rcpp", bufs=2))
    outp = ctx.enter_context(tc.tile_pool(name="outp", bufs=4))
    bigps = ctx.enter_context(tc.tile_pool(name="bigps", bufs=2, space="PSUM"))
    avps = ctx.enter_context(tc.tile_pool(name="avps", bufs=1, space="PSUM"))
    mmps = ctx.enter_context(tc.tile_pool(name="mmps", bufs=1, space="PSUM"))

    # ---- constants / weights (DMA sliced so early compute can start) ----
    bqk_sb = const.tile([P, 2 * DL // P], dt.float32, tag="bqk")
    nc.sync.dma_start(bqk_sb[:], bqk)
    mask_sb = const.tile([P, 4, 1024], MM_DT, tag="mask")
    nc.sync.dma_start(mask_sb[:], maskd)

    # V weights first (needed by emit_v)
    wq_sb = []
    for c in range(NDC):
        t = const.tile([P, 3 * DL], MM_DT, tag=f"wq{c}")
        nc.sync.dma_start(t[:, 2 * DL : 3 * DL], wqkv[c * P : (c + 1) * P, 2 * DL : 3 * DL])
        wq_sb.append(t)
    # x^T in column slices (first 128 columns alone so emit_v(0) starts early)
    xt_sb = []
    for c in range(NDC):
        t = const.tile([P, S], MM_DT, tag=f"xt{c}")
        nc.sync.dma_start(t[:, 0:128], xt[c * P : (c + 1) * P, 0:128])
        xt_sb.append(t)
    for c in range(NDC):
        nc.sync.dma_start(xt_sb[c][:, 128:512], xt[c * P : (c + 1) * P, 128:512])
    # q/k weight columns
    for c in range(NDC):
        nc.sync.dma_start(wq_sb[c][:, 0 : 2 * DL], wqkv[c * P : (c + 1) * P, 0 : 2 * DL])
    # remaining x^T slices
    for q in range(1, 4):
        for c in range(NDC):
            nc.sync.dma_start(
                xt_sb[c][:, q * 512 : (q + 1) * 512],
                xt[c * P : (c + 1) * P, q * 512 : (q + 1) * 512],
            )
    wp_sb = []
    for c in range(DL // P):
        t = const.tile([P, D], MM_DT, tag=f"wp{c}")
        nc.sync.dma_start(t[:], wproj[c * P : (c + 1) * P, :])
        wp_sb.append(t)

    # ---- persistent intermediates ----
    qkvT = const.tile([P, 2 * DL // P, S], MM_DT, tag="qkvT")   # q^T (mt 0-3), k^T (mt 4-7)
    vsb = const.tile([P, NRT, HPC, HD], MM_DT, tag="vsb")       # V natural [row, h, hd]
    yT = const.tile([P, DL // P, S], MM_DT, tag="yT")
    ones64 = const.tile([P, HD], MM_DT, tag="ones64")
    nc.vector.memset(ones64[:], 1.0)

    # ---- phase 1a: V in natural layout [row, (h, hd)] ----
    def emit_v(rt, lead=False):
        # lead-in units borrow the double-buffered attention psum pool so
        # consecutive units pipeline (mmps has a single buffer)
        ps = (
            bigps.tile([P, 1024], dt.float32, tag="wide", name=f"v{rt}")
            if lead
            else mmps.tile([P, 1024], dt.float32, tag="mm", name=f"v{rt}")
        )
        for c in range(NDC):
            nc.tensor.matmul(
                ps[:, 0:512],
                xt_sb[c][:, rt * P : (rt + 1) * P],
                wq_sb[c][:, 2 * DL : 3 * DL],
                start=(c == 0),
                stop=(c == NDC - 1),
            )
        nc.vector.tensor_copy(
            vsb[:, rt, :, :],
            ps[:, 0:512].rearrange("p (h d) -> p h d", d=HD),
        )

    # ---- phase 1b: q^T / k^T m-tiles, rg-paired for weight reuse: the two
    # rg chains share the stationary operand pairwise (LDWEIGHTS dedupe) and
    # live in the two banks of one wide PSUM tile ----
    def qk_unit(mt, rgp, lead=False):
        rg0 = 2 * rgp
        ps = (
            bigps.tile([P, 1024], dt.float32, tag="wide", name=f"qk{mt}_{rg0}")
            if lead
            else mmps.tile([P, 1024], dt.float32, tag="mm", name=f"qk{mt}_{rg0}")
        )
        for c in range(NDC):
            nc.tensor.matmul(
                ps[:, 0:512],
                wq_sb[c][:, mt * P : (mt + 1) * P],
                xt_sb[c][:, rg0 * 512 : (rg0 + 1) * 512],
                start=(c == 0),
                stop=(c == NDC - 1),
            )
            nc.tensor.matmul(
                ps[:, 512:1024],
                wq_sb[c][:, mt * P : (mt + 1) * P],
                xt_sb[c][:, (rg0 + 1) * 512 : (rg0 + 2) * 512],
                start=(c == 0),
                stop=(c == NDC - 1),
            )
        nc.vector.tensor_scalar_add(
            qkvT[:, mt, rg0 * 512 : (rg0 + 2) * 512], ps[:], bqk_sb[:, mt : mt + 1]
        )

    # ---- phase 3: projection, output halves in one wide PSUM tile ----
    def emit_proj(qt):
        ps = mmps.tile([P, 1024], dt.float32, tag="mm", name=f"pj{qt}")
        for c in range(DL // P):
            nc.tensor.matmul(
                ps[:, 0:512],
                yT[:, c, qt * P : (qt + 1) * P],
                wp_sb[c][:, 0:512],
                start=(c == 0),
                stop=(c == DL // P - 1),
            )
            nc.tensor.matmul(
                ps[:, 512:1024],
                yT[:, c, qt * P : (qt + 1) * P],
                wp_sb[c][:, 512:1024],
                start=(c == 0),
                stop=(c == DL // P - 1),
            )
        o = outp.tile([P, 1024], MM_DT, tag="o", name=f"o{qt}")
        nc.vector.tensor_copy(o[:], ps[:])
        nc.sync.dma_start(out_p[qt * P : (qt + 1) * P, :], o[:])

    # ---- phase 2: attention for one head pair ----
    def emit_attention(pr, fillers, fillers_by_qg=None):
        fill_q = list(fillers)

        def fill():
            if fill_q:
                fill_q.pop(0)()

        for qg in range(NQG):
            if fillers_by_qg and qg in fillers_by_qg:
                fill_q.extend(fillers_by_qg[qg])
            nkt = 4 * qg + 4
            nch = (nkt + 3) // 4
            # bank A: av for both heads (col tiles (0,0)/(0,64));
            # bank B: Z broadcast via all-ones stationary, same col tiling
            ava = avps.tile([P, 512], dt.float32, tag="ava", name=f"ava{pr}_{qg}")
            avz = avps.tile([P, 512], dt.float32, tag="avz", name=f"avz{pr}_{qg}")
            pts = {}
            # diagonal tile kt (td = kt-4*qg >= 0) only touches q in
            # [128*td, 512): trim QK/exp/mask/AV/Z to that range
            def lo(kt):
                td = kt - 4 * qg
                return 128 * td if td > 0 else 0

            for ch in range(nch + 1):
                # AV/Z of the previous chunk first: their pt inputs are ready,
                # while this chunk's QKs may still wait on psum-bank reuse
                if ch >= 1:
                    for kt in range(4 * (ch - 1), min(4 * ch, nkt)):
                        pt = pts.pop(kt)
                        ql = lo(kt)
                        nc.tensor.matmul(
                            ava[0:64, ql:512],
                            vsb[:, kt, 2 * pr, :],
                            pt[:, ql:512],
                            start=(kt == 0),
                            stop=(kt == nkt - 1),
                        )
                        nc.tensor.matmul(
                            ava[64:128, ql:512],
                            vsb[:, kt, 2 * pr + 1, :],
                            pt[:, 512 + ql : 1024],
                            start=(kt == 0),
                            stop=(kt == nkt - 1),
                        )
                        nc.tensor.matmul(
                            avz[0:64, ql:512],
                            ones64[:],
                            pt[:, ql:512],
                            start=(kt == 0),
                            stop=(kt == nkt - 1),
                        )
                        nc.tensor.matmul(
                            avz[64:128, ql:512],
                            ones64[:],
                            pt[:, 512 + ql : 1024],
                            start=(kt == 0),
                            stop=(kt == nkt - 1),
                        )
                if ch < nch:
                    for kt in range(4 * ch, min(4 * ch + 4, nkt)):
                        td = kt - 4 * qg
                        ql = lo(kt)
                        wide = bigps.tile([P, 1024], dt.float32, tag="wide", name=f"w{kt%2}")
                        nc.tensor.matmul(
                            wide[:, ql:512],
                            qkvT[0:64, 4 + pr, kt * P : (kt + 1) * P],
                            qkvT[0:64, pr, qg * 512 + ql : (qg + 1) * 512],
                            start=True,
                            stop=True,
                        )
                        nc.tensor.matmul(
                            wide[:, 512 + ql : 1024],
                            qkvT[64:128, 4 + pr, kt * P : (kt + 1) * P],
                            qkvT[64:128, pr, qg * 512 + ql : (qg + 1) * 512],
                            start=True,
                            stop=True,
                        )
                        pt = ptp.tile([P, 1024], MM_DT, tag="pt", name=f"pt{kt%8}")
                        wide2 = wide[:].rearrange("p (s q) -> p s q", s=2)
                        pt2 = pt[:].rearrange("p (s q) -> p s q", s=2)
                        nc.scalar.activation(
                            pt2[:, :, ql:512], wide2[:, :, ql:512], AF.Exp, scale=0.125
                        )
                        if td >= 0:
                            nc.vector.tensor_tensor(
                                pt2[:, :, ql : ql + 128],
                                pt2[:, :, ql : ql + 128],
                                mask_sb[:].rearrange("p t (s q) -> p t s q", s=2)[
                                    :, td, :, ql : ql + 128
                                ],
                                mybir.AluOpType.mult,
                            )
                        pts[kt] = pt
                fill()
            # epilogue: y^T = av * (1/Z); Z already broadcast across partitions
            rcpf = rcpp.tile([P, 512], dt.float32, tag="rcpf", name=f"rcp{pr}_{qg}")
            nc.vector.reciprocal_approx_fast(rcpf[:], avz[:])
            nc.vector.tensor_mul(
                yT[:, pr, qg * 512 : (qg + 1) * 512], ava[:], rcpf[:]
            )
        while fill_q:
            fill_q.pop(0)()

    # ---- emission order ----
    for rt in range(4):
        emit_v(rt, lead=True)
    # q^T/k^T for pair 0, rg 0+1 (enough for qg0/qg1)
    qk_unit(0, 0, lead=True)
    qk_unit(4, 0, lead=True)

    def mk(*fns):
        def go():
            for f in fns:
                f()
        return go

    fillers0 = [
        mk(lambda: qk_unit(0, 1), lambda: qk_unit(4, 1)),
        mk(lambda: emit_v(4), lambda: emit_v(5)),
        mk(lambda: emit_v(6), lambda: emit_v(7)),
        mk(lambda: emit_v(8), lambda: emit_v(9)),
        mk(lambda: emit_v(10), lambda: emit_v(11)),
        mk(lambda: emit_v(12), lambda: emit_v(13)),
        mk(lambda: emit_v(14), lambda: emit_v(15)),
        mk(lambda: qk_unit(1, 0)),
        mk(lambda: qk_unit(5, 0)),
        mk(lambda: qk_unit(1, 1)),
        mk(lambda: qk_unit(5, 1)),
    ]
    emit_attention(0, fillers0)
    for pair in range(1, HPC // 2):
        if pair + 1 < HPC // 2:
            fillers = []
            for rgp in range(2):
                fillers.append(mk(lambda mt=pair + 1, r=rgp: qk_unit(mt, r)))
                fillers.append(mk(lambda mt=4 + pair + 1, r=rgp: qk_unit(mt, r)))
            emit_attention(pair, fillers)
        else:
            # last pair: proj(qt) needs yT for ALL pairs, so qts of q-group g
            # may only be emitted after this pair's qg g epilogue (during qg
            # g+1). qg3's qts drain at the end.
            by_qg = {
                g: [mk(lambda q=qt: emit_proj(q)) for qt in range(4 * (g - 1), 4 * g)]
                for g in range(1, NQG)
            }
            emit_attention(pair, [], by_qg)
    for qt in range(12, NRT):
        emit_proj(qt)


def _build():
    if "nc" in _CACHE:
        return _CACHE["nc"]
    nc = bacc.Bacc("TRN2", target_bir_lowering=False, debug=False, num_devices=NCORES)
    xt = nc.dram_tensor("xt", [D, S], MM_DT, kind="ExternalInput").ap()
    wqkv = nc.dram_tensor("wqkv", [D, 3 * DL], MM_DT, kind="ExternalInput").ap()
    wproj = nc.dram_tensor("wproj", [DL, D], MM_DT, kind="ExternalInput").ap()
    bqk = nc.dram_tensor("bqk", [P, 2 * DL // P], dt.float32, kind="ExternalInput").ap()
    maskd = nc.dram_tensor("maskd", [P, 4, 1024], MM_DT, kind="ExternalInput").ap()
    out_p = nc.dram_tensor("out_p", [S, D], MM_DT, kind="ExternalOutput").ap()

    io = (xt, wqkv, wproj, bqk, maskd, out_p)
    with tile.TileContext(nc) as tc, ExitStack() as ctx:
        _emit(nc, tc, ctx, io)
    nc.compile()
    _CACHE["nc"] = nc
    return nc


def _in_maps(x, w_qkv, b_qkv, w_proj, b_proj):
    x = np.asarray(x, dtype=np.float32)
    w_qkv = np.asarray(w_qkv, dtype=np.float32)
    b_qkv = np.asarray(b_qkv, dtype=np.float32)
    w_proj = np.asarray(w_proj, dtype=np.float32)

    # causal mask for the 4 diagonal-tile alignments, duplicated for the
    # two heads packed side by side: [128, 4, 1024]
    kp = np.arange(P)[:, None, None]
    td = np.arange(4)[None, :, None]
    qf = np.arange(512)[None, None, :]
    maskh = ((P * td + kp) <= qf).astype(NP_MM)
    maskd = np.concatenate([maskh, maskh], axis=2)

    maps = []
    for c in range(NCORES):
        b, half = divmod(c, 2)
        lo, hi = half * DL, (half + 1) * DL
        wq = w_qkv[:, lo:hi]
        wk = w_qkv[:, D + lo : D + hi]
        wv = w_qkv[:, 2 * D + lo : 2 * D + hi]
        wqkv_l = np.concatenate([wq, wk, wv], axis=1).astype(NP_MM)
        bqk_l = np.concatenate([b_qkv[lo:hi], b_qkv[D + lo : D + hi]])
        bqk_t = np.ascontiguousarray(bqk_l.reshape(2 * DL // P, P).T)  # [128, 8]
        maps.append(
            {
                "xt": np.ascontiguousarray(x[b].T).astype(NP_MM),
                "wqkv": wqkv_l,
                "wproj": w_proj[lo:hi, :].astype(NP_MM),
                "bqk": bqk_t,
                "maskd": maskd,
            }
        )
    return maps


def _run(x, w_qkv, b_qkv, w_proj, b_proj, trace=False):
    nc = _build()
    maps = _in_maps(x, w_qkv, b_qkv, w_proj, b_proj)
    res = run_bass_kernel_spmd(nc, maps, list(range(NCORES)), trace=trace)
    b_qkv = np.asarray(b_qkv, dtype=np.float32)
    w_proj = np.asarray(w_proj, dtype=np.float32)
    b_proj = np.asarray(b_proj, dtype=np.float32)
    # V-bias and proj-bias folded on host: softmax rows sum to 1, so
    # y @ wp + bp == y0 @ wp + (bv @ wp + bp)
    bias = b_qkv[2 * D :] @ w_proj + b_proj
    out = np.empty((B, S, D), dtype=np.float32)
    for b in range(B):
        out[b] = (
            res.results[2 * b]["out_p"].astype(np.float32)
            + res.results[2 * b + 1]["out_p"].astype(np.float32)
            + bias
        )
    return out, res


def kernel(x, w_qkv, b_qkv, w_proj, b_proj):
    out, _ = _run(x, w_qkv, b_qkv, w_proj, b_proj)
    return out


# revision 20
# speedup vs baseline: 1.1984x; 1.1984x over previous
"""Causal self-attention (B=4, S=2048, D=1024, H=16) on 8 TRN2 NeuronCores.

Sharding (hybrid batch x heads): core c handles batch b = c//2 and head half
h = c%2 (8 heads, 512 channels).

Per-core pipeline (all matmul data fp16, accumulation fp32 in PSUM):
  qkvT slice = (w_qkv_local^T x^T) via PE, weight-stationary with rg-paired
  reuse;  V in natural [row, (h, hd)] layout.
  Attention per head-pair (even head on partitions 0-63, odd on 64-127):
    QK^T: two 64-contraction matmuls issued as concurrent PE row-tiles
      (tile_position (0,0) / (64,0)) into one two-bank wide PSUM tile.
    exp on ScalarE over the wide [128, 1024] tile (scale=1/8 fused).
    causal mask multiply on GpSimd (diagonal tiles only).
    running softmax denominator: DVE accumulates exp tiles; GpSimd
      partition_all_reduce gives Z broadcast to all partitions; 1/Z =
      exp(-ln Z) on ScalarE (ln+exp share one ACT table set - no reloads).
    AV: two M=64 matmuls as concurrent PE column-tiles ((0,0)/(0,64)),
      accumulating over k tiles; final y^T = av * (1/Z) on DVE.
  proj: y^T-stationary matmuls, rg-paired; b_proj and the V-bias
  contribution are folded in on the host (softmax rows sum to 1).
"""

import numpy as np
import ml_dtypes
from contextlib import ExitStack

import concourse.bass as bass
import concourse.bass_isa as bass_isa
import concourse.mybir as mybir
import concourse.tile as tile
from concourse import bacc
from concourse.bass_utils import run_bass_kernel_spmd

dt = mybir.dt
AF = mybir.ActivationFunctionType

B, S, D, H, HD = 4, 2048, 1024, 16, 64
NCORES = 8
HPC = 8            # heads per core
DL = HPC * HD      # local channel width (512)
P = 128
NQG = S // 512     # q groups of 512          -> 4
NDC = D // P       # D chunks of 128          -> 8
NRT = S // P       # row tiles for V / proj   -> 16

MM_DT = dt.float16
NP_MM = ml_dtypes.float16 if hasattr(ml_dtypes, "float16") else np.float16

_CACHE = {}


def _emit(nc, tc, ctx, io):
    xt, wqkv, wproj, bqk, maskd, out_p = io

    const = ctx.enter_context(tc.tile_pool(name="const", bufs=1))
    ptp = ctx.enter_context(tc.tile_pool(name="ptp", bufs=8))
    rcpp = ctx.enter_context(tc.tile_pool(name="rcpp", bufs=2))
    outp = ctx.enter_context(tc.tile_pool(name="outp", bufs=4))
    bigps = ctx.enter_context(tc.tile_pool(name="bigps", bufs=2, space="PSUM"))
    avps = ctx.enter_context(tc.tile_pool(name="avps", bufs=1, space="PSUM"))
    mmps = ctx.enter_context(tc.tile_pool(name="mmps", bufs=1, space="PSUM"))

    # ---- constants / weights (DMA sliced so early compute can start) ----
    bqk_sb = const.tile([P, 2 * DL // P], dt.float32, tag="bqk")
    nc.sync.dma_start(bqk_sb[:], bqk)
    mask_sb = const.tile([P, 4, 1024], MM_DT, tag="mask")
    nc.sync.dma_start(mask_sb[:], maskd)

    # V weights first (needed by emit_v)
    wq_sb = []
    for c in range(NDC):
        t = const.tile([P, 3 * DL], MM_DT, tag=f"wq{c}")
        nc.sync.dma_start(t[:, 2 * DL : 3 * DL], wqkv[c * P : (c + 1) * P, 2 * DL : 3 * DL])
        wq_sb.append(t)
    # x^T in column slices (first 128 columns alone so emit_v(0) starts early)
    xt_sb = []
    for c in range(NDC):
        t = const.tile([P, S], MM_DT, tag=f"xt{c}")
        nc.sync.dma_start(t[:, 0:128], xt[c * P : (c + 1) * P, 0:128])
        xt_sb.append(t)
    for c in range(NDC):
        nc.sync.dma_start(xt_sb[c][:, 128:512], xt[c * P : (c + 1) * P, 128:512])
    # q/k weight columns
    for c in range(NDC):
        nc.sync.dma_start(wq_sb[c][:, 0 : 2 * DL], wqkv[c * P : (c + 1) * P, 0 : 2 * DL])
    # remaining x^T slices
    for q in range(1, 4):
        for c in range(NDC):
            nc.sync.dma_start(
                xt_sb[c][:, q * 512 : (q + 1) * 512],
                xt[c * P : (c + 1) * P, q * 512 : (q + 1) * 512],
            )
    wp_sb = []
    for c in range(DL // P):
        t = const.tile([P, D], MM_DT, tag=f"wp{c}")
        nc.sync.dma_start(t[:], wproj[c * P : (c + 1) * P, :])
        wp_sb.append(t)

    # ---- persistent intermediates ----
    qkvT = const.tile([P, 2 * DL // P, S], MM_DT, tag="qkvT")   # q^T (mt 0-3), k^T (mt 4-7)
    vsb = const.tile([P, NRT, HPC, HD], MM_DT, tag="vsb")       # V natural [row, h, hd]
    yT = const.tile([P, DL // P, S], MM_DT, tag="yT")
    ones64 = const.tile([P, HD], MM_DT, tag="ones64")
    nc.vector.memset(ones64[:], 1.0)

    # ---- phase 1a: V in natural layout [row, (h, hd)] ----
    def emit_v(rt, lead=False):
        # lead-in units borrow the double-buffered attention psum pool so
        # consecutive units pipeline (mmps has a single buffer)
        ps = (
            bigps.tile([P, 1024], dt.float32, tag="wide", name=f"v{rt}")
            if lead
            else mmps.tile([P, 1024], dt.float32, tag="mm", name=f"v{rt}")
        )
        for c in range(NDC):
            nc.tensor.matmul(
                ps[:, 0:512],
                xt_sb[c][:, rt * P : (rt + 1) * P],
                wq_sb[c][:, 2 * DL : 3 * DL],
                start=(c == 0),
                stop=(c == NDC - 1),
            )
        nc.vector.tensor_copy(
            vsb[:, rt, :, :],
            ps[:, 0:512].rearrange("p (h d) -> p h d", d=HD),
        )

    # ---- phase 1b: q^T / k^T m-tiles, rg-paired for weight reuse: the two
    # rg chains share the stationary operand pairwise (LDWEIGHTS dedupe) and
    # live in the two banks of one wide PSUM tile ----
    def qk_unit(mt, rgp, lead=False):
        rg0 = 2 * rgp
        ps = (
            bigps.tile([P, 1024], dt.float32, tag="wide", name=f"qk{mt}_{rg0}")
            if lead
            else mmps.tile([P, 1024], dt.float32, tag="mm", name=f"qk{mt}_{rg0}")
        )
        for c in range(NDC):
            nc.tensor.matmul(
                ps[:, 0:512],
                wq_sb[c][:, mt * P : (mt + 1) * P],
                xt_sb[c][:, rg0 * 512 : (rg0 + 1) * 512],
                start=(c == 0),
                stop=(c == NDC - 1),
            )
            nc.tensor.matmul(
                ps[:, 512:1024],
                wq_sb[c][:, mt * P : (mt + 1) * P],
                xt_sb[c][:, (rg0 + 1) * 512 : (rg0 + 2) * 512],
                start=(c == 0),
                stop=(c == NDC - 1),
            )
        nc.vector.tensor_scalar_add(
            qkvT[:, mt, rg0 * 512 : (rg0 + 2) * 512], ps[:], bqk_sb[:, mt : mt + 1]
        )

    # ---- phase 3: projection, output halves in one wide PSUM tile ----
    def emit_proj(qt):
        ps = mmps.tile([P, 1024], dt.float32, tag="mm", name=f"pj{qt}")
        for c in range(DL // P):
            nc.tensor.matmul(
                ps[:, 0:512],
                yT[:, c, qt * P : (qt + 1) * P],
                wp_sb[c][:, 0:512],
                start=(c == 0),
                stop=(c == DL // P - 1),
            )
            nc.tensor.matmul(
                ps[:, 512:1024],
                yT[:, c, qt * P : (qt + 1) * P],
                wp_sb[c][:, 512:1024],
                start=(c == 0),
                stop=(c == DL // P - 1),
            )
        o = outp.tile([P, 1024], MM_DT, tag="o", name=f"o{qt}")
        nc.vector.tensor_copy(o[:], ps[:])
        nc.sync.dma_start(out_p[qt * P : (qt + 1) * P, :], o[:])

    # ---- phase 2: attention for one head pair ----
    def emit_attention(pr, fillers, fillers_by_qg=None):
        fill_q = list(fillers)

        def fill():
            if fill_q:
                fill_q.pop(0)()

        for qg in range(NQG):
            if fillers_by_qg and qg in fillers_by_qg:
                fill_q.extend(fillers_by_qg[qg])
            nkt = 4 * qg + 4
            nch = (nkt + 3) // 4
            # bank A: av for both heads (col tiles (0,0)/(0,64));
            # bank B: Z broadcast via all-ones stationary, same col tiling
            ava = avps.tile([P, 512], dt.float32, tag="ava", name=f"ava{pr}_{qg}")
            avz = avps.tile([P, 512], dt.float32, tag="avz", name=f"avz{pr}_{qg}")
            pts = {}
            # diagonal tile kt (td = kt-4*qg >= 0) only touches q in
            # [128*td, 512): trim QK/exp/mask/AV/Z to that range
            def lo(kt):
                td = kt - 4 * qg
                return 128 * td if td > 0 else 0

            for ch in range(nch + 1):
                if ch < nch:
                    for kt in range(4 * ch, min(4 * ch + 4, nkt)):
                        td = kt - 4 * qg
                        ql = lo(kt)
                        wide = bigps.tile([P, 1024], dt.float32, tag="wide", name=f"w{kt%2}")
                        nc.tensor.matmul(
                            wide[:, ql:512],
                            qkvT[0:64, 4 + pr, kt * P : (kt + 1) * P],
                            qkvT[0:64, pr, qg * 512 + ql : (qg + 1) * 512],
                            start=True,
                            stop=True,
                        )
                        nc.tensor.matmul(
                            wide[:, 512 + ql : 1024],
                            qkvT[64:128, 4 + pr, kt * P : (kt + 1) * P],
                            qkvT[64:128, pr, qg * 512 + ql : (qg + 1) * 512],
                            start=True,
                            stop=True,
                        )
                        pt = ptp.tile([P, 1024], MM_DT, tag="pt", name=f"pt{kt%8}")
                        wide2 = wide[:].rearrange("p (s q) -> p s q", s=2)
                        pt2 = pt[:].rearrange("p (s q) -> p s q", s=2)
                        nc.scalar.activation(
                            pt2[:, :, ql:512], wide2[:, :, ql:512], AF.Exp, scale=0.125
                        )
                        if td >= 0:
                            nc.vector.tensor_tensor(
                                pt2[:, :, ql : ql + 128],
                                pt2[:, :, ql : ql + 128],
                                mask_sb[:].rearrange("p t (s q) -> p t s q", s=2)[
                                    :, td, :, ql : ql + 128
                                ],
                                mybir.AluOpType.mult,
                            )
                        pts[kt] = pt
                if ch >= 1:
                    for kt in range(4 * (ch - 1), min(4 * ch, nkt)):
                        pt = pts.pop(kt)
                        ql = lo(kt)
                        nc.tensor.matmul(
                            ava[0:64, ql:512],
                            vsb[:, kt, 2 * pr, :],
                            pt[:, ql:512],
                            start=(kt == 0),
                            stop=(kt == nkt - 1),
                        )
                        nc.tensor.matmul(
                            ava[64:128, ql:512],
                            vsb[:, kt, 2 * pr + 1, :],
                            pt[:, 512 + ql : 1024],
                            start=(kt == 0),
                            stop=(kt == nkt - 1),
                        )
                        nc.tensor.matmul(
                            avz[0:64, ql:512],
                            ones64[:],
                            pt[:, ql:512],
                            start=(kt == 0),
                            stop=(kt == nkt - 1),
                        )
                        nc.tensor.matmul(
                            avz[64:128, ql:512],
                            ones64[:],
                            pt[:, 512 + ql : 1024],
                            start=(kt == 0),
                            stop=(kt == nkt - 1),
                        )
                fill()
            # epilogue: y^T = av * (1/Z); Z already broadcast across partitions
            rcpf = rcpp.tile([P, 512], dt.float32, tag="rcpf", name=f"rcp{pr}_{qg}")
            nc.vector.reciprocal_approx_fast(rcpf[:], avz[:])
            nc.vector.tensor_mul(
                yT[:, pr, qg * 512 : (qg + 1) * 512], ava[:], rcpf[:]
            )
        while fill_q:
            fill_q.pop(0)()

    # ---- emission order ----
    for rt in range(4):
        emit_v(rt, lead=True)
    # q^T/k^T for pair 0, rg 0+1 (enough for qg0/qg1)
    qk_unit(0, 0, lead=True)
    qk_unit(4, 0, lead=True)

    def mk(*fns):
        def go():
            for f in fns:
                f()
        return go

    fillers0 = [
        mk(lambda: qk_unit(0, 1), lambda: qk_unit(4, 1)),
        mk(lambda: emit_v(4), lambda: emit_v(5)),
        mk(lambda: emit_v(6), lambda: emit_v(7)),
        mk(lambda: emit_v(8), lambda: emit_v(9)),
        mk(lambda: emit_v(10), lambda: emit_v(11)),
        mk(lambda: emit_v(12), lambda: emit_v(13)),
        mk(lambda: emit_v(14), lambda: emit_v(15)),
        mk(lambda: qk_unit(1, 0)),
        mk(lambda: qk_unit(5, 0)),
        mk(lambda: qk_unit(1, 1)),
        mk(lambda: qk_unit(5, 1)),
    ]
    emit_attention(0, fillers0)
    for pair in range(1, HPC // 2):
        if pair + 1 < HPC // 2:
            fillers = []
            for rgp in range(2):
                fillers.append(mk(lambda mt=pair + 1, r=rgp: qk_unit(mt, r)))
                fillers.append(mk(lambda mt=4 + pair + 1, r=rgp: qk_unit(mt, r)))
            emit_attention(pair, fillers)
        else:
            # last pair: proj(qt) needs yT for ALL pairs, so qts of q-group g
            # may only be emitted after this pair's qg g epilogue (during qg
            # g+1). qg3's qts drain at the end.
            by_qg = {
                g: [mk(lambda q=qt: emit_proj(q)) for qt in range(4 * (g - 1), 4 * g)]
                for g in range(1, NQG)
            }
            emit_attention(pair, [], by_qg)
    for qt in range(12, NRT):
        emit_proj(qt)


def _build():
    if "nc" in _CACHE:
        return _CACHE["nc"]
    nc = bacc.Bacc("TRN2", target_bir_lowering=False, debug=False, num_devices=NCORES)
    xt = nc.dram_tensor("xt", [D, S], MM_DT, kind="ExternalInput").ap()
    wqkv = nc.dram_tensor("wqkv", [D, 3 * DL], MM_DT, kind="ExternalInput").ap()
    wproj = nc.dram_tensor("wproj", [DL, D], MM_DT, kind="ExternalInput").ap()
    bqk = nc.dram_tensor("bqk", [P, 2 * DL // P], dt.float32, kind="ExternalInput").ap()
    maskd = nc.dram_tensor("maskd", [P, 4, 1024], MM_DT, kind="ExternalInput").ap()
    out_p = nc.dram_tensor("out_p", [S, D], MM_DT, kind="ExternalOutput").ap()

    io = (xt, wqkv, wproj, bqk, maskd, out_p)
    with tile.TileContext(nc) as tc, ExitStack() as ctx:
        _emit(nc, tc, ctx, io)
    nc.compile()
    _CACHE["nc"] = nc
    return nc


def _in_maps(x, w_qkv, b_qkv, w_proj, b_proj):
    x = np.asarray(x, dtype=np.float32)
    w_qkv = np.asarray(w_qkv, dtype=np.float32)
    b_qkv = np.asarray(b_qkv, dtype=np.float32)
    w_proj = np.asarray(w_proj, dtype=np.float32)

    # causal mask for the 4 diagonal-tile alignments, duplicated for the
    # two heads packed side by side: [128, 4, 1024]
    kp = np.arange(P)[:, None, None]
    td = np.arange(4)[None, :, None]
    qf = np.arange(512)[None, None, :]
    maskh = ((P * td + kp) <= qf).astype(NP_MM)
    maskd = np.concatenate([maskh, maskh], axis=2)

    maps = []
    for c in range(NCORES):
        b, half = divmod(c, 2)
        lo, hi = half * DL, (half + 1) * DL
        wq = w_qkv[:, lo:hi]
        wk = w_qkv[:, D + lo : D + hi]
        wv = w_qkv[:, 2 * D + lo : 2 * D + hi]
        wqkv_l = np.concatenate([wq, wk, wv], axis=1).astype(NP_MM)
        bqk_l = np.concatenate([b_qkv[lo:hi], b_qkv[D + lo : D + hi]])
        bqk_t = np.ascontiguousarray(bqk_l.reshape(2 * DL // P, P).T)  # [128, 8]
        maps.append(
            {
                "xt": np.ascontiguousarray(x[b].T).astype(NP_MM),
                "wqkv": wqkv_l,
                "wproj": w_proj[lo:hi, :].astype(NP_MM),
                "bqk": bqk_t,
                "maskd": maskd,
            }
        )
    return maps


def _run(x, w_qkv, b_qkv, w_proj, b_proj, trace=False):
    nc = _build()
    maps = _in_maps(x, w_qkv, b_qkv, w_proj, b_proj)
    res = run_bass_kernel_spmd(nc, maps, list(range(NCORES)), trace=trace)
    b_qkv = np.asarray(b_qkv, dtype=np.float32)
    w_proj = np.asarray(w_proj, dtype=np.float32)
    b_proj = np.asarray(b_proj, dtype=np.float32)
    # V-bias and proj-bias folded on host: softmax rows sum to 1, so
    # y @ wp + bp == y0 @ wp + (bv @ wp + bp)
    bias = b_qkv[2 * D :] @ w_proj + b_proj
    out = np.empty((B, S, D), dtype=np.float32)
    for b in range(B):
        out[b] = (
            res.results[2 * b]["out_p"].astype(np.float32)
            + res.results[2 * b + 1]["out_p"].astype(np.float32)
            + bias
        )
    return out, res


def kernel(x, w_qkv, b_qkv, w_proj, b_proj):
    out, _ = _run(x, w_qkv, b_qkv, w_proj, b_proj)
    return out
